# revision 12
# baseline (speedup 1.0000x reference)
"""Trainium2 Bass kernel for nn_DiffusionModel1d (batched tridiagonal solve).

Math: the reference solves A(K) u = h^2 * f with K = exp(alpha) and
A = tridiag(-K_j, K_j + K_{j+1}, -K_{j+1})  (row 0 diag 2K_0 + K_1,
row n-1 = K_{n-1} (u_{n-1} - u_{n-2})).  This matrix factors exactly as

    A = B^T diag(K_0..K_{n-1}) B + K_0 e_0 e_0^T

with B the unit lower-bidiagonal first-difference operator, so by
Sherman-Morrison (and since B^{-1} is cumsum, B^{-T} rev-cumsum):

    z = cumsum( rev_cumsum(h^2 f) / K );   u = z - z_0 / 2

Folding the -z_0/2 correction into a halved first weight gives

    u = cumsum( w' * exp(-alpha) ),  w' = rev_cumsum(h^2 f), w'_0 *= 0.5

i.e. one exp (ACT), one multiply and one hardware prefix-scan (DVE) per
element.  Sharding: pure data parallel over batch across the 8 cores.

Written in raw Bass (explicit semaphores, standalone wait instructions):
the walrus build in this container rejects any compute instruction whose
sync_info carries more than one semaphore wait, which rules out
Tile-generated scheduling here.  Sync scheme:
 - s_dve: monotone counter incremented by every DVE op; all RAW/WAR
   hazards against DVE outputs wait on the producer's counter value.
 - s_act: exp completion counter (ACT).
 - per-buffer-slot DMA semaphores: consecutive DMAs from one engine
   land on different queues and complete out of order, so a cumulative
   counter cannot identify one transfer; slot semaphores are sound
   because the next DMA on a slot only issues after the slot drained.
"""

import numpy as np

import concourse.bass as bass
import concourse.mybir as mybir
from concourse.bass_utils import run_bass_kernel_spmd

F32 = mybir.dt.float32
ALU = mybir.AluOpType
ACT_EXP = mybir.ActivationFunctionType.Exp

N_CORES = 8
B_FULL = 16384
M = 1024
N = M - 1               # 1023 unknowns
P = 128                 # SBUF partitions
B_SHARD = B_FULL // N_CORES
NB = 4                  # pipeline buffers per stage
H2 = 1.0 / float(N * N)

N_SETUP = 5             # DVE ops in the W' setup chain


def _mult_idx(t):       # s_dve value after mult(t) completes
    return N_SETUP + 2 * t + 1


def _scan_idx(t):       # s_dve value after scan(t) completes
    return N_SETUP + 2 * t + 2


def _build(b_shard: int = B_SHARD, rep: int = 1) -> bass.Bass:
    """Build the kernel.  rep > 1 repeats the whole per-core pipeline rep
    times inside one NEFF (same data, same output) — used by test.py to
    measure pure HW kernel time as a wall-clock slope over rep."""
    nc = bass.Bass("TRN2")
    alpha = nc.dram_tensor("alpha", [b_shard, M], F32, kind="ExternalInput")
    f_rhs = nc.dram_tensor("f_rhs", [N], F32, kind="ExternalInput")
    out = nc.dram_tensor("u", [b_shard, N], F32, kind="ExternalOutput")

    ntiles = b_shard // P
    alpha_t = alpha[:].rearrange("(t p) m -> t p m", p=P)
    out_t = out[:].rearrange("(t p) m -> t p m", p=P)

    # DRAM view of f broadcast across all 128 partitions
    f_ap = f_rhs[:][None, :]
    f_bcast = bass.AP(tensor=f_ap.tensor, offset=f_ap.offset,
                      ap=[[0, P]] + [list(d) for d in f_ap.ap[1:]])

    import contextlib
    with contextlib.ExitStack() as ctx:
        fb = ctx.enter_context(nc.sbuf_tensor([P, N], F32))    # f -> g = h2*f
        csum = ctx.enter_context(nc.sbuf_tensor([P, N], F32))  # cumsum of g
        w = ctx.enter_context(nc.sbuf_tensor([P, N], F32))     # weight row
        A = ctx.enter_context(nc.sbuf_tensor([P, NB, N], F32))  # alpha tiles
        R = ctx.enter_context(nc.sbuf_tensor([P, NB, N], F32))  # exp(-alpha)
        X = ctx.enter_context(nc.sbuf_tensor([P, NB, N], F32))  # w * R
        U = ctx.enter_context(nc.sbuf_tensor([P, NB, N], F32))  # cumsum(X)
        s_fw = ctx.enter_context(nc.semaphore("s_fw"))
        s_load = [ctx.enter_context(nc.semaphore(f"s_load{i}"))
                  for i in range(NB)]
        s_store = [ctx.enter_context(nc.semaphore(f"s_store{i}"))
                   for i in range(NB)]
        s_act = ctx.enter_context(nc.semaphore("s_act"))
        s_dve = ctx.enter_context(nc.semaphore("s_dve"))
        block = ctx.enter_context(nc.Block())

        @block.sync
        def _(sync):
            # one-time: fetch f (replicated to all partitions)
            sync.dma_start(out=fb[:, :], in_=f_bcast).then_inc(s_fw, 16)
            # alpha tile loads
            for t in range(rep * ntiles):
                b = t % NB
                if t >= NB:
                    # A[:, b] is free once exp(t-NB) consumed it
                    sync.wait_ge(s_act, t - NB + 1)
                sync.dma_start(out=A[:, b, :],
                               in_=alpha_t[t % ntiles, :, 0:N]).then_inc(
                                   s_load[b], 16)

        @block.scalar
        def _(scalar):
            for t in range(rep * ntiles):
                b = t % NB
                scalar.wait_ge(s_load[b], 16 * (t // NB + 1))
                if t >= NB:
                    # R[:, b] is free once mult(t-NB) consumed it
                    scalar.wait_ge(s_dve, _mult_idx(t - NB))
                nc.scalar.activation(R[:, b, :], A[:, b, :], ACT_EXP,
                                     scale=-1.0).then_inc(s_act, 1)

        @block.vector
        def _(vector):
            # ---- one-time W' setup (5 chained DVE ops, idx 1..N_SETUP) ----
            vector.wait_ge(s_fw, 16)
            nc.vector.tensor_scalar_mul(fb[:, :], fb[:, :],
                                        float(H2)).then_inc(s_dve, 1)   # 1
            vector.wait_ge(s_dve, 1)
            nc.vector.tensor_tensor_scan(csum[:, :], fb[:, :], fb[:, :], 0.0,
                                         ALU.add,
                                         ALU.bypass).then_inc(s_dve, 1)  # 2
            vector.wait_ge(s_dve, 2)
            # w = (-csum + g) + csum[:, -1]  == rev_cumsum(g)
            nc.vector.scalar_tensor_tensor(w[:, :], csum[:, :], -1.0, fb[:, :],
                                           ALU.mult,
                                           ALU.add).then_inc(s_dve, 1)   # 3
            vector.wait_ge(s_dve, 3)
            nc.vector.tensor_scalar_add(w[:, :], w[:, :],
                                        csum[:, N - 1:N]).then_inc(s_dve, 1)
            vector.wait_ge(s_dve, 4)
            nc.vector.tensor_scalar_mul(w[:, 0:1], w[:, 0:1],
                                        0.5).then_inc(s_dve, 1)          # 5
            # ---- per-tile multiply + prefix scan ----
            for t in range(rep * ntiles):
                b = t % NB
                vector.wait_ge(s_act, t + 1)
                # needs W' done and (for t>=NB) X[:, b] drained by scan(t-NB)
                vector.wait_ge(s_dve,
                               N_SETUP if t < NB else _scan_idx(t - NB))
                nc.vector.scalar_tensor_tensor(X[:, b, :], R[:, b, :], 0.0,
                                               w[:, :], ALU.bypass,
                                               ALU.mult).then_inc(s_dve, 1)
                if t >= NB:
                    # U[:, b] is free once store(t-NB) finished
                    vector.wait_ge(s_store[b], 16 * ((t - NB) // NB + 1))
                vector.wait_ge(s_dve, _mult_idx(t))
                nc.vector.tensor_tensor_scan(U[:, b, :], X[:, b, :],
                                             X[:, b, :], 0.0, ALU.add,
                                             ALU.bypass).then_inc(s_dve, 1)

        @block.gpsimd
        def _(gpsimd):
            for t in range(rep * ntiles):
                b = t % NB
                gpsimd.wait_ge(s_dve, _scan_idx(t))
                gpsimd.dma_start(out=out_t[t % ntiles],
                                 in_=U[:, b, :]).then_inc(s_store[b], 16)

    return nc


_cache: dict = {}


def kernel(alpha: np.ndarray, f_rhs: np.ndarray) -> np.ndarray:
    assert alpha.shape == (B_FULL, M) and f_rhs.shape == (N,)
    if "nc" not in _cache:
        _cache["nc"] = _build()
    nc = _cache["nc"]

    f32 = np.ascontiguousarray(f_rhs, dtype=np.float32)
    in_maps = [
        {
            "alpha": np.ascontiguousarray(alpha[i * B_SHARD:(i + 1) * B_SHARD]),
            "f_rhs": f32,
        }
        for i in range(N_CORES)
    ]
    # The axon-tunneled devices occasionally come up wedged from a prior
    # aborted process and fail the first dispatch with
    # NRT_EXEC_UNIT_UNRECOVERABLE; the condition self-heals, so retry.
    last_exc = None
    for _ in range(3):
        try:
            res = run_bass_kernel_spmd(nc, in_maps, list(range(N_CORES)))
            break
        except Exception as exc:  # noqa: BLE001
            last_exc = exc
            import time as _time
            _time.sleep(5)
            try:
                import jax
                jax.clear_caches()
                jax.clear_backends()
            except Exception:  # noqa: BLE001
                pass
    else:
        raise last_exc
    return np.concatenate([res.results[i]["u"] for i in range(N_CORES)], axis=0)
